# revision 17
# baseline (speedup 1.0000x reference)
"""Trainium2 Bass kernel for nn_AdaptiveGeometryModule.

Data-parallel over batch across 8 NeuronCores. Key algebraic facts used:
  - The seq-len-1 multi-head attention is the identity on v, so
      refined = balanced @ (Wv @ proj_w) + (bv @ proj_w + proj_b)
    with Wv = qkv_w[:, 2D:3D], bv = qkv_b[2D:3D].
  - balanced = (softmax(-dist/temp) @ anch) * mlp_weight, so with
    U = exp(logits), rinv = 1/rowsum(U), w = sigmoid(mlp),
      refined = ([U * rinv * w | 1] @ [anch @ Wv @ proj_w ; b_comb])
  - center_loss rows = ||f||^2 - 2 f.refined + ||refined||^2, with
      f.refined = sum_a U'[a] * (f @ McatT)[a]   (extra matmul columns)
      ||refined||^2 = U' @ (Mcat Mcat^T) @ U'^T  (tiny Gram quadratic form)
No collectives needed: each core emits a scalar loss partial; host sums.
"""

import numpy as np
import ml_dtypes

import concourse.bass as bass
import concourse.mybir as mybir
import concourse.tile as tile
from concourse.bass_utils import run_bass_kernel_spmd

F32 = mybir.dt.float32
F32R = mybir.dt.float32r
BF16 = mybir.dt.bfloat16
AX = mybir.AxisListType
OP = mybir.AluOpType
ACTF = mybir.ActivationFunctionType

B, D, A, H = 16384, 512, 10, 8
NCORES = 8
BL = B // NCORES          # 2048 rows per core
NT = BL // 128            # 16 tiles of 128 rows
NCH = NT // 4             # 4 chunks of 512 rows
DC = D // 128             # 4 d-chunks
NCOLS = A + A + 1         # 21: [-2*anch | McatT(11)] columns

_CACHE = {}


def _split_waits(nc, maxw=1):
    """walrus on this image rejects instructions with more than a couple
    of inline sync-wait commands (Tile's tail drain aggregates one per
    engine domain). Split excess waits onto same-engine NOPs placed
    immediately before the instruction — identical semantics, since the
    engine stalls at the NOP's wait either way."""
    f = nc.m.functions[0]
    cur = nc.cur_bb.bb
    for bb in f.blocks:
        il = list(bb.instructions)
        need = any(i.sync_info and i.sync_info.on_wait
                   and len(i.sync_info.on_wait) > maxw for i in il)
        if not need:
            continue
        out = []
        for inst in il:
            si = inst.sync_info
            waits = list(si.on_wait) if si and si.on_wait else []
            if len(waits) > maxw:
                extra, keep = waits[:-maxw], waits[-maxw:]
                for i in range(0, len(extra), maxw):
                    b = nc.engines[inst.engine].nop(hint="waitsplit")
                    n = b.ins
                    cl = cur.instructions
                    assert cl[-1].name == n.name
                    cur.instructions = cl[:-1]
                    n.sync_info = mybir.SyncInfo(
                        on_wait=extra[i:i + maxw], on_update=[])
                    out.append(n)
                inst.sync_info = mybir.SyncInfo(
                    on_wait=keep,
                    on_update=list(si.on_update)
                    if si and si.on_update else [])
            out.append(inst)
        bb.instructions = out


def build_nc():
    nc = bass.Bass("TRN2", target_bir_lowering=False, debug=False,
                   num_devices=NCORES)

    # ---- dram parameters (per-core shard + replicated constants) ----
    d_feat = nc.dram_tensor("features", [BL, D], F32, kind="ExternalInput")
    d_w1 = nc.dram_tensor("c_w1", [D, 256], BF16, kind="ExternalInput")
    d_w2 = nc.dram_tensor("c_w2", [128, 2], BF16, kind="ExternalInput")
    d_b1 = nc.dram_tensor("c_b1", [128, 2], F32, kind="ExternalInput")
    d_nb2 = nc.dram_tensor("c_nb2", [128, 1], F32, kind="ExternalInput")
    d_anchcat = nc.dram_tensor("c_anchcat", [D, 32], BF16,
                               kind="ExternalInput")
    d_na2c = nc.dram_tensor("c_na2col", [32, 1], F32, kind="ExternalInput")
    d_mcatp = nc.dram_tensor("c_mcatp", [128, D], BF16, kind="ExternalInput")
    d_gcatp = nc.dram_tensor("c_gcatp", [128, A + 1], BF16,
                             kind="ExternalInput")
    d_id16 = nc.dram_tensor("c_id16", [128, 128], BF16, kind="ExternalInput")
    d_id32 = nc.dram_tensor("c_id32", [128, 128], F32, kind="ExternalInput")
    d_onesr = nc.dram_tensor("c_onesr", [1, 128], F32, kind="ExternalInput")
    d_onesc = nc.dram_tensor("c_onesc", [128, 1], F32, kind="ExternalInput")
    d_ones10 = nc.dram_tensor("c_ones10", [A, 1], F32, kind="ExternalInput")
    d_mask = nc.dram_tensor("c_mask", [A, A], F32, kind="ExternalInput")
    d_arT = nc.dram_tensor("c_arT", [D, A], F32, kind="ExternalInput")
    d_arTm2 = nc.dram_tensor("c_arTm2", [D, A], F32, kind="ExternalInput")
    d_anch = nc.dram_tensor("anchors", [A, D], F32, kind="ExternalInput")
    d_clw = nc.dram_tensor("center_loss_weight", [1, 1], F32,
                           kind="ExternalInput")

    d_ref = nc.dram_tensor("refined", [BL, D], F32, kind="ExternalOutput")
    d_att = nc.dram_tensor("attention", [BL, A], F32, kind="ExternalOutput")
    d_loss = nc.dram_tensor("loss", [1, 1], F32, kind="ExternalOutput")

    from contextlib import ExitStack
    with tile.TileContext(nc) as tc, ExitStack() as ctx:
        pc = ctx.enter_context(tc.tile_pool(name="consts", bufs=1))
        p_f16 = ctx.enter_context(tc.tile_pool(name="f16", bufs=NT))
        p_ftT = ctx.enter_context(tc.tile_pool(name="ftT", bufs=NT))
        p_rel = ctx.enter_context(tc.tile_pool(name="relu", bufs=2 * NCH))
        p_ref = ctx.enter_context(tc.tile_pool(name="refsb", bufs=2))
        p_sm = ctx.enter_context(tc.tile_pool(name="smalls", bufs=NT))
        p_chn = ctx.enter_context(tc.tile_pool(name="chunks", bufs=NCH))
        p_scr = ctx.enter_context(tc.tile_pool(name="scratch", bufs=2))

        # PSUM pools (8 banks)
        ps_tr = ctx.enter_context(
            tc.tile_pool(name="ps_tr", bufs=1, space="PSUM"))
        ps_hid = ctx.enter_context(
            tc.tile_pool(name="ps_hid", bufs=1, space="PSUM"))
        ps_ref = ctx.enter_context(
            tc.tile_pool(name="ps_ref", bufs=2, space="PSUM"))
        ps_dot = ctx.enter_context(
            tc.tile_pool(name="ps_dot", bufs=1, space="PSUM"))
        ps_sm = ctx.enter_context(
            tc.tile_pool(name="ps_sm", bufs=1, space="PSUM"))
        ps_pv = ctx.enter_context(
            tc.tile_pool(name="ps_pv", bufs=2, space="PSUM"))

        def csb(dram, shape, dt, name):
            t = pc.tile(shape, dt, tag=name)
            nc.sync.dma_start(t[:], dram[:, :])
            return t

        w1sb = [csb(d_w1[128 * c:128 * (c + 1), :], [128, 256], BF16,
                    f"w1_{c}") for c in range(DC)]
        w2sb = csb(d_w2, [128, 2], BF16, "w2")
        b1sb = csb(d_b1, [128, 2], F32, "b1")
        nb2sb = csb(d_nb2, [128, 1], F32, "nb2")
        acsb = [csb(d_anchcat[128 * c:128 * (c + 1), :], [128, 32], BF16,
                    f"ac_{c}") for c in range(DC)]
        na2csb = csb(d_na2c, [32, 1], F32, "na2c")
        mcpsb = csb(d_mcatp, [128, D], BF16, "mcatp")
        gcpsb = csb(d_gcatp, [128, A + 1], BF16, "gcatp")
        id16 = csb(d_id16, [128, 128], BF16, "id16")
        id32 = csb(d_id32, [128, 128], F32, "id32")
        onesr = csb(d_onesr, [1, 128], F32, "onesr")
        onesc = csb(d_onesc, [128, 1], F32, "onesc")
        ones10 = csb(d_ones10, [A, 1], F32, "ones10")
        masksb = csb(d_mask, [A, A], F32, "mask")
        arTsb = [csb(d_arT[128 * c:128 * (c + 1), :], [128, A], F32,
                     f"arT_{c}") for c in range(DC)]
        arTm2sb = [csb(d_arTm2[128 * c:128 * (c + 1), :], [128, A], F32,
                       f"arTm2_{c}") for c in range(DC)]
        anchsb = csb(d_anch, [A, D], F32, "anchrow")
        clwsb = csb(d_clw, [1, 1], F32, "clw")

        css_all = pc.tile([128, NCH], F32, tag="css_all")

        # persistent U'cat tiles (bf16), ones column preset; padded U'catT
        uc_all = []
        for t in range(NT):
            uc = p_sm.tile([128, A + 1], BF16, tag="uc")
            nc.gpsimd.memset(uc[:, A:A + 1], 1.0)
            uc_all.append(uc)
        uTp = []
        for k in range(NCH):
            u = p_chn.tile([128, 512], BF16, tag="uTp")
            nc.gpsimd.memset(u[:], 0.0)
            uTp.append(u)

        # ---- load features (SWDGE cast fp32 -> bf16) ----
        f16 = []
        for t in range(NT):
            ft = p_f16.tile([128, D], BF16, tag="f16")
            nc.gpsimd.dma_start(ft[:], d_feat[128 * t:128 * (t + 1), :])
            f16.append(ft)

        # ---- ACT phase 1: all norm^2 squares (single table load) ----
        norm2 = []
        for k in range(NCH):
            n2 = p_chn.tile([128, 4], F32, tag="norm2")
            norm2.append(n2)
            for j in range(4):
                scr = p_scr.tile([128, D], BF16, tag="sqscr")
                nc.scalar.activation(scr[:], f16[4 * k + j][:], ACTF.Square,
                                     accum_out=n2[:, j:j + 1])

        # ---- featT via PE transposes (bf16 PSUM) + DVE evac ----
        featT = {}
        for k in range(NCH):
            for c in range(DC):
                ptr = ps_tr.tile([128, 512], BF16, tag="tr")
                for j in range(4):
                    nc.tensor.transpose(
                        ptr[:, 128 * j:128 * (j + 1)],
                        f16[4 * k + j][:, 128 * c:128 * (c + 1)], id16[:])
                fT = p_ftT.tile([128, 512], BF16, tag="ftT")
                nc.vector.tensor_copy(fT[:], ptr[:])
                featT[(c, k)] = fT

        for k in range(NCH):
            tiles = [4 * k + j for j in range(4)]
            n2 = norm2[k]

            # -- MLP hiddenT; relu evac on DVE (bias add + max 0) --
            relus = []
            for h in range(2):
                ph = ps_hid.tile([128, 512], F32, tag="hid")
                for c in range(DC):
                    nc.tensor.matmul(
                        ph[:], w1sb[c][:, 128 * h:128 * (h + 1)],
                        featT[(c, k)][:], start=(c == 0), stop=(c == DC - 1))
                rl = p_rel.tile([128, 512], BF16, tag="relu")
                nc.vector.tensor_scalar(rl[:], ph[:], b1sb[:, h:h + 1], 0.0,
                                        OP.add, OP.max)
                relus.append(rl)

            # -- wl = relu @ w2 -> [1,512]; repack to [128,4] via 4 DMAs --
            pwl = ps_hid.tile([1, 512], F32, tag="hid")
            for h in range(2):
                nc.tensor.matmul(pwl[:], w2sb[:, h:h + 1], relus[h][:],
                                 start=(h == 0), stop=(h == 1))
            wlsb = p_chn.tile([1, 512], F32, tag="wlsb")
            nc.vector.tensor_copy(wlsb[:], pwl[:])
            wraw = p_chn.tile([128, 4], F32, tag="wraw")
            for j in range(4):
                nc.gpsimd.dma_start(wraw[:, j:j + 1],
                                    wlsb[0:1, 128 * j:128 * (j + 1)])

            # -- T-dots: anchcat stationary [128,32]; += handled via bias --
            pdT = ps_dot.tile([32, 512], F32, tag="dotsT")
            for c in range(DC):
                nc.tensor.matmul(pdT[:], acsb[c][:], featT[(c, k)][:],
                                 start=(c == 0), stop=(c == DC - 1))
            dcsb = p_chn.tile([32, 512], BF16, tag="dotscat")
            nc.vector.tensor_scalar(dcsb[:], pdT[:], na2csb[:, 0:1], None,
                                    OP.add)
            prd = ps_dot.tile([128, 128], BF16, tag="dotsT")
            for j in range(4):
                nc.tensor.transpose(prd[:, 32 * j:32 * (j + 1)],
                                    dcsb[:, 128 * j:128 * (j + 1)],
                                    id16[0:32, 0:32])
            rdall = p_chn.tile([128, 128], BF16, tag="rdall")
            nc.vector.tensor_copy(rdall[:], prd[:])
            rds = [rdall[:, 32 * j:32 * (j + 1)] for j in range(4)]

            # -- ACT sqrt phase: temperature + distances --
            tmp = p_chn.tile([128, 4], F32, tag="tempb")
            nc.scalar.activation(tmp[:], n2[:], ACTF.Sqrt)
            dists = []
            for j in range(4):
                dist = p_sm.tile([128, A], F32, tag="dist")
                nc.scalar.activation(dist[:], rds[j][:, 0:A], ACTF.Sqrt,
                                     bias=n2[:, j:j + 1])
                dists.append(dist)
            rtemp = p_chn.tile([128, 4], F32, tag="rtemp")
            nc.vector.reciprocal(rtemp[:], tmp[:])
            nrt = p_chn.tile([128, 4], F32, tag="nrt")
            nc.vector.tensor_scalar(nrt[:], rtemp[:], -1.0, None, OP.mult)

            # -- ACT exp phase: U = exp(-dist/temp); sigmoid via exp --
            rowsum = p_chn.tile([128, 4], F32, tag="rowsum")
            Us = []
            for j in range(4):
                U = p_sm.tile([128, A], F32, tag="U")
                nc.scalar.activation(U[:], dists[j][:], ACTF.Exp,
                                     scale=nrt[:, j:j + 1],
                                     accum_out=rowsum[:, j:j + 1])
                Us.append(U)
            e4 = p_chn.tile([128, 4], F32, tag="e4")
            nc.scalar.activation(e4[:], wraw[:], ACTF.Exp, scale=-1.0,
                                 bias=nb2sb[:, 0:1])
            t4 = p_chn.tile([128, 4], F32, tag="t4")
            nc.vector.tensor_scalar(t4[:], e4[:], 1.0, None, OP.add)
            wtil = p_chn.tile([128, 4], F32, tag="wtil")
            nc.vector.reciprocal(wtil[:], t4[:])
            rinv = p_chn.tile([128, 4], F32, tag="rinv")
            nc.vector.reciprocal(rinv[:], rowsum[:])

            # -- U'cat, attention out, U'catT (padded) --
            puT = ps_sm.tile([A + 1, 512], BF16, tag="sm")
            attk = p_chn.tile([128, 4 * A], F32, tag="attk")
            for j, t in enumerate(tiles):
                nc.gpsimd.tensor_scalar(attk[:, A * j:A * (j + 1)],
                                         Us[j][:], rinv[:, j:j + 1],
                                         None, OP.mult)
                uc = uc_all[t]
                nc.gpsimd.tensor_scalar(uc[:, 0:A], Us[j][:],
                                        rinv[:, j:j + 1], wtil[:, j:j + 1],
                                        OP.mult, OP.mult)
                nc.tensor.transpose(puT[:, 128 * j:128 * (j + 1)], uc[:],
                                    id16[:])
            nc.vector.tensor_copy(uTp[k][0:A + 1, :], puT[:])
            nc.sync.dma_start(
                d_att[512 * k:512 * (k + 1), :].rearrange(
                    "(j p) a -> p j a", p=128), attk[:])

            # -- refined + V + center pieces --
            scrF = p_chn.tile([128, 4 * (A + 1)], F32, tag="scrF")
            scrQ = p_chn.tile([128, 4 * (A + 1)], F32, tag="scrQ")
            refch = p_ref.tile([128, 4 * D], F32, tag="refch")
            for j, t in enumerate(tiles):
                pr = ps_ref.tile([128, D], F32, tag="ref")
                nc.tensor.matmul(pr[:], uTp[k][:, 128 * j:128 * (j + 1)],
                                 mcpsb[:], start=True, stop=True)
                pv = ps_pv.tile([128, A + 1], F32, tag="pv")
                nc.tensor.matmul(pv[:], uTp[k][:, 128 * j:128 * (j + 1)],
                                 gcpsb[:], start=True, stop=True)
                nc.gpsimd.tensor_tensor(
                    scrF[:, (A + 1) * j:(A + 1) * (j + 1)], uc_all[t][:],
                    rds[j][:, A:2 * A + 1], OP.mult)
                nc.vector.tensor_tensor(
                    scrQ[:, (A + 1) * j:(A + 1) * (j + 1)], uc_all[t][:],
                    pv[:], OP.mult)
                nc.vector.tensor_copy(refch[:, D * j:D * (j + 1)], pr[:])
            nc.sync.dma_start(
                d_ref[512 * k:512 * (k + 1), :].rearrange(
                    "(j p) d -> p j d", p=128), refch[:])
            frs = p_chn.tile([128, 1], F32, tag="frs")
            nc.vector.tensor_reduce(frs[:], scrF[:], AX.X, OP.add)
            qqs = p_chn.tile([128, 1], F32, tag="qqs")
            nc.vector.tensor_reduce(qqs[:], scrQ[:], AX.X, OP.add)
            n2s = p_chn.tile([128, 1], F32, tag="n2s")
            nc.vector.tensor_reduce(n2s[:], n2[:], AX.X, OP.add)
            t1 = p_chn.tile([128, 1], F32, tag="t1")
            nc.vector.tensor_scalar(t1[:], frs[:], -2.0, None, OP.mult)
            nc.vector.tensor_tensor(t1[:], t1[:], qqs[:], OP.add)
            nc.vector.tensor_tensor(css_all[:, k:k + 1], t1[:], n2s[:],
                                    OP.add)

        # ---------------- loss tail ----------------
        csum = p_sm.tile([128, 1], F32, tag="csum")
        nc.vector.tensor_reduce(csum[:], css_all[:], AX.X, OP.add)
        pcs = ps_sm.tile([1, 1], F32, tag="sm")
        nc.tensor.matmul(pcs[:], csum[:], onesc[:], start=True, stop=True)

        na2r = p_sm.tile([A, 1], F32, tag="na2r")
        scrA = p_scr.tile([A, D], F32, tag="scrA")
        nc.scalar.activation(scrA[:], anchsb[:], ACTF.Square,
                             accum_out=na2r[:])
        pna = ps_sm.tile([1, A], F32, tag="sm")
        nc.tensor.transpose(pna[:], na2r[:], id32[0:A, 0:A])
        na2row = p_sm.tile([1, A], F32, tag="na2row")
        nc.vector.tensor_copy(na2row[:], pna[:])
        pg = ps_sm.tile([A, A], F32, tag="sm")
        nc.tensor.matmul(pg[:], onesr[:, 0:A], na2row[:], start=True,
                         stop=False)
        for c in range(DC):
            nc.tensor.matmul(pg[:], arTm2sb[c][:], arTsb[c][:],
                             start=False, stop=(c == DC - 1))
        pd2 = p_sm.tile([A, A], F32, tag="pd2")
        nc.vector.tensor_scalar(pd2[:], pg[:], na2r[:], 0.0, OP.add, OP.max)
        pdm = p_sm.tile([A, A], F32, tag="pdm")
        nc.scalar.activation(pdm[:], pd2[:], ACTF.Sqrt)
        scrm = p_scr.tile([A, A], F32, tag="scrm")
        pdsum = p_sm.tile([A, 1], F32, tag="pdsum")
        nc.vector.tensor_tensor(scrm[:], pdm[:], masksb[:], OP.mult)
        nc.vector.tensor_reduce(pdsum[:], scrm[:], AX.X, OP.add)
        ppd = ps_sm.tile([1, 1], F32, tag="sm")
        nc.tensor.matmul(ppd[:], pdsum[:], ones10[:], start=True, stop=True)

        lossA = p_sm.tile([1, 1], F32, tag="lossA")
        nc.vector.tensor_scalar(lossA[:], pcs[:], clwsb[:, 0:1], 1.0 / B,
                                OP.mult, OP.mult)
        lossB = p_sm.tile([1, 1], F32, tag="lossB")
        nc.vector.tensor_scalar(lossB[:], ppd[:], -0.1 / 45.0 / NCORES, None,
                                OP.mult)
        lsb = p_sm.tile([1, 1], F32, tag="lsb")
        nc.vector.tensor_tensor(lsb[:], lossA[:], lossB[:], OP.add)
        nc.sync.dma_start(d_loss[:, :], lsb[:])

    _split_waits(nc)
    return nc


def host_consts(anchors, pos_embedding, qkv_w, qkv_b, proj_w, proj_b,
                w1, b1, w2, b2, center_loss_weight):
    f32, bf16 = np.float32, ml_dtypes.bfloat16
    anch = (anchors + pos_embedding[0]).astype(f32)          # [A, D]
    Wv = qkv_w[:, 2 * D:3 * D].astype(f32)
    bv = qkv_b[2 * D:3 * D].astype(f32)
    Mcomb = (anch @ Wv @ proj_w).astype(f32)                 # [A, D]
    bcomb = (bv @ proj_w + proj_b).astype(f32)               # [D]
    Mcat = np.concatenate([Mcomb, bcomb[None]], 0).astype(f32)  # [11, D]
    Gcat = (Mcat @ Mcat.T).astype(f32)                       # [11, 11]
    # anchcat [D, 32]: cols 0..9 = -2*anch^T, 10..20 = Mcat^T, 21..31 = 0
    anchcat = np.zeros((D, 32), f32)
    anchcat[:, :A] = -2.0 * anch.T
    anchcat[:, A:2 * A + 1] = Mcat.T
    na2col = np.zeros((32, 1), f32)
    na2col[:A, 0] = np.sum(anch * anch, axis=1)
    mcatp = np.zeros((128, D), f32)
    mcatp[:A + 1] = Mcat
    gcatp = np.zeros((128, A + 1), f32)
    gcatp[:A + 1] = Gcat
    mask = np.triu(np.ones((A, A), f32), k=1)
    c = {
        "c_w1": w1.astype(bf16),
        "c_w2": np.ascontiguousarray(w2[:, 0].reshape(2, 128).T).astype(bf16),
        "c_b1": np.ascontiguousarray(b1.reshape(2, 128).T).astype(f32),
        "c_nb2": np.full((128, 1), -np.float32(b2[0]), f32),
        "c_anchcat": anchcat.astype(bf16),
        "c_na2col": na2col,
        "c_mcatp": mcatp.astype(bf16),
        "c_gcatp": gcatp.astype(bf16),
        "c_id16": np.eye(128, dtype=bf16),
        "c_id32": np.eye(128, dtype=f32),
        "c_onesr": np.ones((1, 128), f32),
        "c_onesc": np.ones((128, 1), f32),
        "c_ones10": np.ones((A, 1), f32),
        "c_mask": mask,
        "c_arT": np.ascontiguousarray(anchors.T).astype(f32),
        "c_arTm2": np.ascontiguousarray(-2.0 * anchors.T).astype(f32),
        "anchors": anchors.astype(f32),
        "center_loss_weight":
            np.full((1, 1), np.float32(center_loss_weight), f32),
    }
    return c


def kernel(**inputs):
    inputs = {k: np.asarray(v) for k, v in inputs.items()}
    consts = host_consts(
        inputs["anchors"], inputs["pos_embedding"], inputs["qkv_w"],
        inputs["qkv_b"], inputs["proj_w"], inputs["proj_b"], inputs["w1"],
        inputs["b1"], inputs["w2"], inputs["b2"],
        inputs["center_loss_weight"])
    feats = inputs["features"].astype(np.float32)
    in_maps = []
    for i in range(NCORES):
        m = dict(consts)
        m["features"] = np.ascontiguousarray(feats[i * BL:(i + 1) * BL])
        in_maps.append(m)

    if "nc" not in _CACHE:
        _CACHE["nc"] = build_nc()
    res = run_bass_kernel_spmd(_CACHE["nc"], in_maps,
                               core_ids=list(range(NCORES)))
    outs = res.results
    refined = np.concatenate([outs[i]["refined"] for i in range(NCORES)], 0)
    attention = np.concatenate([outs[i]["attention"] for i in range(NCORES)],
                               0)
    total = np.float32(sum(float(outs[i]["loss"][0, 0])
                           for i in range(NCORES)))
    return refined, total, attention


# revision 18
# speedup vs baseline: 1.1371x; 1.1371x over previous
"""Trainium2 Bass kernel for nn_AdaptiveGeometryModule.

Data-parallel over batch across 8 NeuronCores. Key algebraic facts used:
  - The seq-len-1 multi-head attention is the identity on v, so
      refined = balanced @ (Wv @ proj_w) + (bv @ proj_w + proj_b)
    with Wv = qkv_w[:, 2D:3D], bv = qkv_b[2D:3D].
  - balanced = (softmax(-dist/temp) @ anch) * mlp_weight, so with
    U = exp(logits), rinv = 1/rowsum(U), w = sigmoid(mlp),
      refined = ([U * rinv * w | 1] @ [anch @ Wv @ proj_w ; b_comb])
  - center_loss rows = ||f||^2 - 2 f.refined + ||refined||^2, with
      f.refined = sum_a U'[a] * (f @ McatT)[a]   (extra matmul columns)
      ||refined||^2 = U' @ (Mcat Mcat^T) @ U'^T  (tiny Gram quadratic form)
No collectives needed: each core emits a scalar loss partial; host sums.
"""

import numpy as np
import ml_dtypes

import concourse.bass as bass
import concourse.mybir as mybir
import concourse.tile as tile
from concourse.bass_utils import run_bass_kernel_spmd

F32 = mybir.dt.float32
F32R = mybir.dt.float32r
BF16 = mybir.dt.bfloat16
AX = mybir.AxisListType
OP = mybir.AluOpType
ACTF = mybir.ActivationFunctionType

B, D, A, H = 16384, 512, 10, 8
NCORES = 8
BL = B // NCORES          # 2048 rows per core
NT = BL // 128            # 16 tiles of 128 rows
NCH = NT // 4             # 4 chunks of 512 rows
DC = D // 128             # 4 d-chunks
NCOLS = A + A + 1         # 21: [-2*anch | McatT(11)] columns

_CACHE = {}


def _split_waits(nc, maxw=1):
    """walrus on this image rejects instructions with more than a couple
    of inline sync-wait commands (Tile's tail drain aggregates one per
    engine domain). Split excess waits onto same-engine NOPs placed
    immediately before the instruction — identical semantics, since the
    engine stalls at the NOP's wait either way."""
    f = nc.m.functions[0]
    cur = nc.cur_bb.bb
    for bb in f.blocks:
        il = list(bb.instructions)
        need = any(i.sync_info and i.sync_info.on_wait
                   and len(i.sync_info.on_wait) > maxw for i in il)
        if not need:
            continue
        out = []
        for inst in il:
            si = inst.sync_info
            waits = list(si.on_wait) if si and si.on_wait else []
            if len(waits) > maxw:
                extra, keep = waits[:-maxw], waits[-maxw:]
                for i in range(0, len(extra), maxw):
                    b = nc.engines[inst.engine].nop(hint="waitsplit")
                    n = b.ins
                    cl = cur.instructions
                    assert cl[-1].name == n.name
                    cur.instructions = cl[:-1]
                    n.sync_info = mybir.SyncInfo(
                        on_wait=extra[i:i + maxw], on_update=[])
                    out.append(n)
                inst.sync_info = mybir.SyncInfo(
                    on_wait=keep,
                    on_update=list(si.on_update)
                    if si and si.on_update else [])
            out.append(inst)
        bb.instructions = out


def build_nc():
    nc = bass.Bass("TRN2", target_bir_lowering=False, debug=False,
                   num_devices=NCORES)

    # ---- dram parameters (per-core shard + replicated constants) ----
    d_feat = nc.dram_tensor("features", [BL, D], F32, kind="ExternalInput")
    d_w1 = nc.dram_tensor("c_w1", [D, 256], BF16, kind="ExternalInput")
    d_w2 = nc.dram_tensor("c_w2", [128, 2], BF16, kind="ExternalInput")
    d_b1 = nc.dram_tensor("c_b1", [128, 2], F32, kind="ExternalInput")
    d_nb2 = nc.dram_tensor("c_nb2", [128, 1], F32, kind="ExternalInput")
    d_anchcat = nc.dram_tensor("c_anchcat", [D, 32], BF16,
                               kind="ExternalInput")
    d_na2c = nc.dram_tensor("c_na2col", [32, 1], F32, kind="ExternalInput")
    d_mcatp = nc.dram_tensor("c_mcatp", [128, D], BF16, kind="ExternalInput")
    d_gcatp = nc.dram_tensor("c_gcatp", [128, A + 1], BF16,
                             kind="ExternalInput")
    d_id16 = nc.dram_tensor("c_id16", [128, 128], BF16, kind="ExternalInput")
    d_id32 = nc.dram_tensor("c_id32", [128, 128], F32, kind="ExternalInput")
    d_onesr = nc.dram_tensor("c_onesr", [1, 128], F32, kind="ExternalInput")
    d_onesc = nc.dram_tensor("c_onesc", [128, 1], F32, kind="ExternalInput")
    d_ones10 = nc.dram_tensor("c_ones10", [A, 1], F32, kind="ExternalInput")
    d_mask = nc.dram_tensor("c_mask", [A, A], F32, kind="ExternalInput")
    d_arT = nc.dram_tensor("c_arT", [D, A], F32, kind="ExternalInput")
    d_arTm2 = nc.dram_tensor("c_arTm2", [D, A], F32, kind="ExternalInput")
    d_anch = nc.dram_tensor("anchors", [A, D], F32, kind="ExternalInput")
    d_clw = nc.dram_tensor("center_loss_weight", [1, 1], F32,
                           kind="ExternalInput")

    d_ref = nc.dram_tensor("refined", [BL, D], F32, kind="ExternalOutput")
    d_att = nc.dram_tensor("attention", [BL, A], F32, kind="ExternalOutput")
    d_loss = nc.dram_tensor("loss", [1, 1], F32, kind="ExternalOutput")

    from contextlib import ExitStack
    with tile.TileContext(nc) as tc, ExitStack() as ctx:
        pc = ctx.enter_context(tc.tile_pool(name="consts", bufs=1))
        p_f16 = ctx.enter_context(tc.tile_pool(name="f16", bufs=NT))
        p_ftT = ctx.enter_context(tc.tile_pool(name="ftT", bufs=NT))
        p_rel = ctx.enter_context(tc.tile_pool(name="relu", bufs=2 * NCH))
        p_ref = ctx.enter_context(tc.tile_pool(name="refsb", bufs=2))
        p_sm = ctx.enter_context(tc.tile_pool(name="smalls", bufs=NT))
        p_chn = ctx.enter_context(tc.tile_pool(name="chunks", bufs=NCH))
        p_scr = ctx.enter_context(tc.tile_pool(name="scratch", bufs=2))

        # PSUM pools (8 banks)
        ps_tr = ctx.enter_context(
            tc.tile_pool(name="ps_tr", bufs=1, space="PSUM"))
        ps_hid = ctx.enter_context(
            tc.tile_pool(name="ps_hid", bufs=1, space="PSUM"))
        ps_ref = ctx.enter_context(
            tc.tile_pool(name="ps_ref", bufs=2, space="PSUM"))
        ps_dot = ctx.enter_context(
            tc.tile_pool(name="ps_dot", bufs=1, space="PSUM"))
        ps_sm = ctx.enter_context(
            tc.tile_pool(name="ps_sm", bufs=1, space="PSUM"))
        ps_pv = ctx.enter_context(
            tc.tile_pool(name="ps_pv", bufs=2, space="PSUM"))

        def csb(dram, shape, dt, name):
            t = pc.tile(shape, dt, tag=name)
            nc.sync.dma_start(t[:], dram[:, :])
            return t

        w1sb = [csb(d_w1[128 * c:128 * (c + 1), :], [128, 256], BF16,
                    f"w1_{c}") for c in range(DC)]
        w2sb = csb(d_w2, [128, 2], BF16, "w2")
        b1sb = csb(d_b1, [128, 2], F32, "b1")
        nb2sb = csb(d_nb2, [128, 1], F32, "nb2")
        acsb = [csb(d_anchcat[128 * c:128 * (c + 1), :], [128, 32], BF16,
                    f"ac_{c}") for c in range(DC)]
        na2csb = csb(d_na2c, [32, 1], F32, "na2c")
        mcpsb = csb(d_mcatp, [128, D], BF16, "mcatp")
        gcpsb = csb(d_gcatp, [128, A + 1], BF16, "gcatp")
        id16 = csb(d_id16, [128, 128], BF16, "id16")
        id32 = csb(d_id32, [128, 128], F32, "id32")
        onesr = csb(d_onesr, [1, 128], F32, "onesr")
        onesc = csb(d_onesc, [128, 1], F32, "onesc")
        ones10 = csb(d_ones10, [A, 1], F32, "ones10")
        masksb = csb(d_mask, [A, A], F32, "mask")
        arTsb = [csb(d_arT[128 * c:128 * (c + 1), :], [128, A], F32,
                     f"arT_{c}") for c in range(DC)]
        arTm2sb = [csb(d_arTm2[128 * c:128 * (c + 1), :], [128, A], F32,
                       f"arTm2_{c}") for c in range(DC)]
        anchsb = csb(d_anch, [A, D], F32, "anchrow")
        clwsb = csb(d_clw, [1, 1], F32, "clw")

        css_all = pc.tile([128, NCH], F32, tag="css_all")

        # persistent U'cat tiles (bf16), ones column preset; padded U'catT
        uc_all = []
        for t in range(NT):
            uc = p_sm.tile([128, A + 1], BF16, tag="uc")
            nc.gpsimd.memset(uc[:, A:A + 1], 1.0)
            uc_all.append(uc)
        uTp = []
        for k in range(NCH):
            u = p_chn.tile([128, 512], BF16, tag="uTp")
            nc.gpsimd.memset(u[:], 0.0)
            uTp.append(u)

        # ---- load features (SWDGE cast fp32 -> bf16) ----
        f16 = []
        for t in range(NT):
            ft = p_f16.tile([128, D], BF16, tag="f16")
            nc.gpsimd.dma_start(ft[:], d_feat[128 * t:128 * (t + 1), :])
            f16.append(ft)

        # ---- ACT phase 1: all norm^2 squares (single table load) ----
        norm2 = []
        for k in range(NCH):
            n2 = p_chn.tile([128, 4], F32, tag="norm2")
            norm2.append(n2)
            for j in range(4):
                scr = p_scr.tile([128, D], BF16, tag="sqscr")
                nc.scalar.activation(scr[:], f16[4 * k + j][:], ACTF.Square,
                                     accum_out=n2[:, j:j + 1])

        # ---- featT via PE transposes (bf16 PSUM) + DVE evac ----
        featT = {}
        for k in range(NCH):
            for c in range(DC):
                ptr = ps_tr.tile([128, 512], BF16, tag="tr")
                for j in range(4):
                    nc.tensor.transpose(
                        ptr[:, 128 * j:128 * (j + 1)],
                        f16[4 * k + j][:, 128 * c:128 * (c + 1)], id16[:])
                fT = p_ftT.tile([128, 512], BF16, tag="ftT")
                nc.vector.tensor_copy(fT[:], ptr[:])
                featT[(c, k)] = fT

        state = {}

        def front_chunk(k):
            tiles = [4 * k + j for j in range(4)]
            n2 = norm2[k]

            # -- MLP hiddenT; relu evac on DVE (bias add + max 0) --
            relus = []
            for h in range(2):
                ph = ps_hid.tile([128, 512], F32, tag="hid")
                for c in range(DC):
                    nc.tensor.matmul(
                        ph[:], w1sb[c][:, 128 * h:128 * (h + 1)],
                        featT[(c, k)][:], start=(c == 0), stop=(c == DC - 1))
                rl = p_rel.tile([128, 512], BF16, tag="relu")
                nc.vector.tensor_scalar(rl[:], ph[:], b1sb[:, h:h + 1], 0.0,
                                        OP.add, OP.max)
                relus.append(rl)

            # -- wl = relu @ w2 -> [1,512]; repack to [128,4] --
            pwl = ps_hid.tile([1, 512], F32, tag="hid")
            for h in range(2):
                nc.tensor.matmul(pwl[:], w2sb[:, h:h + 1], relus[h][:],
                                 start=(h == 0), stop=(h == 1))
            wlsb = p_chn.tile([1, 512], F32, tag="wlsb")
            nc.vector.tensor_copy(wlsb[:], pwl[:])
            wraw = p_chn.tile([128, 4], F32, tag="wraw")
            for j in range(4):
                nc.gpsimd.dma_start(wraw[:, j:j + 1],
                                    wlsb[0:1, 128 * j:128 * (j + 1)])

            # -- T-dots: anchcat stationary --
            pdT = ps_dot.tile([32, 512], F32, tag="dotsT")
            for c in range(DC):
                nc.tensor.matmul(pdT[:], acsb[c][:], featT[(c, k)][:],
                                 start=(c == 0), stop=(c == DC - 1))
            dcsb = p_chn.tile([32, 512], BF16, tag="dotscat")
            nc.vector.tensor_scalar(dcsb[:], pdT[:], na2csb[:, 0:1], None,
                                    OP.add)
            prd = ps_dot.tile([128, 128], BF16, tag="dotsT")
            for j in range(4):
                nc.tensor.transpose(prd[:, 32 * j:32 * (j + 1)],
                                    dcsb[:, 128 * j:128 * (j + 1)],
                                    id16[0:32, 0:32])
            rdall = p_chn.tile([128, 128], BF16, tag="rdall")
            nc.vector.tensor_copy(rdall[:], prd[:])
            rds = [rdall[:, 32 * j:32 * (j + 1)] for j in range(4)]

            # -- ACT sqrt phase --
            tmp = p_chn.tile([128, 4], F32, tag="tempb")
            nc.scalar.activation(tmp[:], n2[:], ACTF.Sqrt)
            dists = []
            for j in range(4):
                dist = p_sm.tile([128, A], F32, tag="dist")
                nc.scalar.activation(dist[:], rds[j][:, 0:A], ACTF.Sqrt,
                                     bias=n2[:, j:j + 1])
                dists.append(dist)
            rtemp = p_chn.tile([128, 4], F32, tag="rtemp")
            nc.vector.reciprocal(rtemp[:], tmp[:])
            nrt = p_chn.tile([128, 4], F32, tag="nrt")
            nc.vector.tensor_scalar(nrt[:], rtemp[:], -1.0, None, OP.mult)

            # -- ACT exp phase --
            rowsum = p_chn.tile([128, 4], F32, tag="rowsum")
            Us = []
            for j in range(4):
                U = p_sm.tile([128, A], F32, tag="U")
                nc.scalar.activation(U[:], dists[j][:], ACTF.Exp,
                                     scale=nrt[:, j:j + 1],
                                     accum_out=rowsum[:, j:j + 1])
                Us.append(U)
            e4 = p_chn.tile([128, 4], F32, tag="e4")
            nc.scalar.activation(e4[:], wraw[:], ACTF.Exp, scale=-1.0,
                                 bias=nb2sb[:, 0:1])
            t4 = p_chn.tile([128, 4], F32, tag="t4")
            nc.vector.tensor_scalar(t4[:], e4[:], 1.0, None, OP.add)
            wtil = p_chn.tile([128, 4], F32, tag="wtil")
            nc.vector.reciprocal(wtil[:], t4[:])
            rinv = p_chn.tile([128, 4], F32, tag="rinv")
            nc.vector.reciprocal(rinv[:], rowsum[:])
            state[k] = (tiles, n2, rds, Us, rinv, wtil)

        def tail_chunk(k):
            tiles, n2, rds, Us, rinv, wtil = state.pop(k)
            puT = ps_sm.tile([A + 1, 512], BF16, tag="sm")
            attk = p_chn.tile([128, 4 * A], F32, tag="attk")
            for j, t in enumerate(tiles):
                nc.gpsimd.tensor_scalar(attk[:, A * j:A * (j + 1)],
                                        Us[j][:], rinv[:, j:j + 1],
                                        None, OP.mult)
                uc = uc_all[t]
                nc.vector.tensor_scalar(uc[:, 0:A], Us[j][:],
                                        rinv[:, j:j + 1], wtil[:, j:j + 1],
                                        OP.mult, OP.mult)
                nc.tensor.transpose(puT[:, 128 * j:128 * (j + 1)], uc[:],
                                    id16[:])
            nc.vector.tensor_copy(uTp[k][0:A + 1, :], puT[:])
            nc.sync.dma_start(
                d_att[512 * k:512 * (k + 1), :].rearrange(
                    "(j p) a -> p j a", p=128), attk[:])

            # -- refined + V + center pieces --
            scrF = p_chn.tile([128, 4 * (A + 1)], F32, tag="scrF")
            scrQ = p_chn.tile([128, 4 * (A + 1)], F32, tag="scrQ")
            refch = p_ref.tile([128, 4 * D], F32, tag="refch")
            for j, t in enumerate(tiles):
                pr = ps_ref.tile([128, D], F32, tag="ref")
                nc.tensor.matmul(pr[:], uTp[k][:, 128 * j:128 * (j + 1)],
                                 mcpsb[:], start=True, stop=True)
                pv = ps_pv.tile([128, A + 1], F32, tag="pv")
                nc.tensor.matmul(pv[:], uTp[k][:, 128 * j:128 * (j + 1)],
                                 gcpsb[:], start=True, stop=True)
                nc.gpsimd.tensor_tensor(
                    scrF[:, (A + 1) * j:(A + 1) * (j + 1)], uc_all[t][:],
                    rds[j][:, A:2 * A + 1], OP.mult)
                nc.vector.tensor_tensor(
                    scrQ[:, (A + 1) * j:(A + 1) * (j + 1)], uc_all[t][:],
                    pv[:], OP.mult)
                nc.vector.tensor_copy(refch[:, D * j:D * (j + 1)], pr[:])
            nc.sync.dma_start(
                d_ref[512 * k:512 * (k + 1), :].rearrange(
                    "(j p) d -> p j d", p=128), refch[:])
            frs = p_chn.tile([128, 1], F32, tag="frs")
            nc.vector.tensor_reduce(frs[:], scrF[:], AX.X, OP.add)
            qqs = p_chn.tile([128, 1], F32, tag="qqs")
            nc.vector.tensor_reduce(qqs[:], scrQ[:], AX.X, OP.add)
            n2s = p_chn.tile([128, 1], F32, tag="n2s")
            nc.vector.tensor_reduce(n2s[:], n2[:], AX.X, OP.add)
            t1 = p_chn.tile([128, 1], F32, tag="t1")
            nc.vector.tensor_scalar(t1[:], frs[:], -2.0, None, OP.mult)
            nc.vector.tensor_tensor(t1[:], t1[:], qqs[:], OP.add)
            nc.vector.tensor_tensor(css_all[:, k:k + 1], t1[:], n2s[:],
                                    OP.add)

        # software-pipelined emission: front runs one chunk ahead of tail
        front_chunk(0)
        front_chunk(1)
        tail_chunk(0)
        front_chunk(2)
        tail_chunk(1)
        front_chunk(3)
        tail_chunk(2)
        tail_chunk(3)

        # ---------------- loss tail ----------------
        csum = p_sm.tile([128, 1], F32, tag="csum")
        nc.vector.tensor_reduce(csum[:], css_all[:], AX.X, OP.add)
        pcs = ps_sm.tile([1, 1], F32, tag="sm")
        nc.tensor.matmul(pcs[:], csum[:], onesc[:], start=True, stop=True)

        na2r = p_sm.tile([A, 1], F32, tag="na2r")
        scrA = p_scr.tile([A, D], F32, tag="scrA")
        nc.scalar.activation(scrA[:], anchsb[:], ACTF.Square,
                             accum_out=na2r[:])
        pna = ps_sm.tile([1, A], F32, tag="sm")
        nc.tensor.transpose(pna[:], na2r[:], id32[0:A, 0:A])
        na2row = p_sm.tile([1, A], F32, tag="na2row")
        nc.vector.tensor_copy(na2row[:], pna[:])
        pg = ps_sm.tile([A, A], F32, tag="sm")
        nc.tensor.matmul(pg[:], onesr[:, 0:A], na2row[:], start=True,
                         stop=False)
        for c in range(DC):
            nc.tensor.matmul(pg[:], arTm2sb[c][:], arTsb[c][:],
                             start=False, stop=(c == DC - 1))
        pd2 = p_sm.tile([A, A], F32, tag="pd2")
        nc.vector.tensor_scalar(pd2[:], pg[:], na2r[:], 0.0, OP.add, OP.max)
        pdm = p_sm.tile([A, A], F32, tag="pdm")
        nc.scalar.activation(pdm[:], pd2[:], ACTF.Sqrt)
        scrm = p_scr.tile([A, A], F32, tag="scrm")
        pdsum = p_sm.tile([A, 1], F32, tag="pdsum")
        nc.vector.tensor_tensor(scrm[:], pdm[:], masksb[:], OP.mult)
        nc.vector.tensor_reduce(pdsum[:], scrm[:], AX.X, OP.add)
        ppd = ps_sm.tile([1, 1], F32, tag="sm")
        nc.tensor.matmul(ppd[:], pdsum[:], ones10[:], start=True, stop=True)

        lossA = p_sm.tile([1, 1], F32, tag="lossA")
        nc.vector.tensor_scalar(lossA[:], pcs[:], clwsb[:, 0:1], 1.0 / B,
                                OP.mult, OP.mult)
        lossB = p_sm.tile([1, 1], F32, tag="lossB")
        nc.vector.tensor_scalar(lossB[:], ppd[:], -0.1 / 45.0 / NCORES, None,
                                OP.mult)
        lsb = p_sm.tile([1, 1], F32, tag="lsb")
        nc.vector.tensor_tensor(lsb[:], lossA[:], lossB[:], OP.add)
        nc.sync.dma_start(d_loss[:, :], lsb[:])

    _split_waits(nc)
    return nc


def host_consts(anchors, pos_embedding, qkv_w, qkv_b, proj_w, proj_b,
                w1, b1, w2, b2, center_loss_weight):
    f32, bf16 = np.float32, ml_dtypes.bfloat16
    anch = (anchors + pos_embedding[0]).astype(f32)          # [A, D]
    Wv = qkv_w[:, 2 * D:3 * D].astype(f32)
    bv = qkv_b[2 * D:3 * D].astype(f32)
    Mcomb = (anch @ Wv @ proj_w).astype(f32)                 # [A, D]
    bcomb = (bv @ proj_w + proj_b).astype(f32)               # [D]
    Mcat = np.concatenate([Mcomb, bcomb[None]], 0).astype(f32)  # [11, D]
    Gcat = (Mcat @ Mcat.T).astype(f32)                       # [11, 11]
    # anchcat [D, 32]: cols 0..9 = -2*anch^T, 10..20 = Mcat^T, 21..31 = 0
    anchcat = np.zeros((D, 32), f32)
    anchcat[:, :A] = -2.0 * anch.T
    anchcat[:, A:2 * A + 1] = Mcat.T
    na2col = np.zeros((32, 1), f32)
    na2col[:A, 0] = np.sum(anch * anch, axis=1)
    mcatp = np.zeros((128, D), f32)
    mcatp[:A + 1] = Mcat
    gcatp = np.zeros((128, A + 1), f32)
    gcatp[:A + 1] = Gcat
    mask = np.triu(np.ones((A, A), f32), k=1)
    c = {
        "c_w1": w1.astype(bf16),
        "c_w2": np.ascontiguousarray(w2[:, 0].reshape(2, 128).T).astype(bf16),
        "c_b1": np.ascontiguousarray(b1.reshape(2, 128).T).astype(f32),
        "c_nb2": np.full((128, 1), -np.float32(b2[0]), f32),
        "c_anchcat": anchcat.astype(bf16),
        "c_na2col": na2col,
        "c_mcatp": mcatp.astype(bf16),
        "c_gcatp": gcatp.astype(bf16),
        "c_id16": np.eye(128, dtype=bf16),
        "c_id32": np.eye(128, dtype=f32),
        "c_onesr": np.ones((1, 128), f32),
        "c_onesc": np.ones((128, 1), f32),
        "c_ones10": np.ones((A, 1), f32),
        "c_mask": mask,
        "c_arT": np.ascontiguousarray(anchors.T).astype(f32),
        "c_arTm2": np.ascontiguousarray(-2.0 * anchors.T).astype(f32),
        "anchors": anchors.astype(f32),
        "center_loss_weight":
            np.full((1, 1), np.float32(center_loss_weight), f32),
    }
    return c


def kernel(**inputs):
    inputs = {k: np.asarray(v) for k, v in inputs.items()}
    consts = host_consts(
        inputs["anchors"], inputs["pos_embedding"], inputs["qkv_w"],
        inputs["qkv_b"], inputs["proj_w"], inputs["proj_b"], inputs["w1"],
        inputs["b1"], inputs["w2"], inputs["b2"],
        inputs["center_loss_weight"])
    feats = inputs["features"].astype(np.float32)
    in_maps = []
    for i in range(NCORES):
        m = dict(consts)
        m["features"] = np.ascontiguousarray(feats[i * BL:(i + 1) * BL])
        in_maps.append(m)

    if "nc" not in _CACHE:
        _CACHE["nc"] = build_nc()
    res = run_bass_kernel_spmd(_CACHE["nc"], in_maps,
                               core_ids=list(range(NCORES)))
    outs = res.results
    refined = np.concatenate([outs[i]["refined"] for i in range(NCORES)], 0)
    attention = np.concatenate([outs[i]["attention"] for i in range(NCORES)],
                               0)
    total = np.float32(sum(float(outs[i]["loss"][0, 0])
                           for i in range(NCORES)))
    return refined, total, attention
